# revision 8
# baseline (speedup 1.0000x reference)
"""ContactLoss Trainium2 kernel (8 NeuronCores, batch data-parallel).

Math: all three losses only need per-hand-vertex and per-obj-vertex MIN
squared distances (the reference's argmin+gather+norm equals sqrt(min d2)),
followed by tanh/sqrt pointwise ops and masked means.

Per core (4 batches):
  d2 tile [128 obj (partitions), 778 hand (free)] computed on PE via K=5
  augmented matmul: rows(lhsT) = [o_x, o_y, o_z, yy+BIG*invalid, 1],
  rows(rhs) = [-2h_x, -2h_y, -2h_z, 1, xx]  ->  d2 = -2 o.h + yyM + xx.
  ACT drains PSUM -> SBUF f16 (enables DVE 2x modes), DVE does:
    - per-obj-tile row-min (minoh) via 3D tensor_reduce over tile groups
    - cross-tile elementwise-min tree (minho fold), then PE transposes +
      small reduces give the partition-axis min.
  tanh(sqrt(.)) on the tiny minima vectors, masked partial sums out.
Host: shard/augment inputs, sum partial numerators, divide by mask counts.
"""

import sys
from contextlib import ExitStack

import numpy as np

sys.path.insert(0, "/opt/trn_rl_repo")

import concourse.bass as bass  # noqa: E402
import concourse.mybir as mybir  # noqa: E402
import concourse.tile as tile  # noqa: E402
from concourse import bacc  # noqa: E402
from concourse.bass_utils import run_bass_kernel_spmd  # noqa: E402
from concourse.masks import make_identity  # noqa: E402

B, NH, NO = 32, 778, 3 * 8192 // 3  # 32, 778, 8192
NO = 8192
NCORES = 8
BPC = B // NCORES  # batches per core
T = NO // 128  # 64 obj tiles per batch
G = 8  # obj tiles per slab group
NG = T // G  # 8 groups per batch
HC = (NH + 127) // 128  # 7 hand column-chunks for the transpose stage
# Mask offset for invalid obj slots. Must dominate any real d2 (<= ~2 here)
# while staying finite in f16 (max 65504) so no inf/NaN enters the pipeline.
BIG = np.float32(60000.0)
# Coordinate pre-scale: d2 values land in f16's normal range (>=6.1e-5)
# even for ~1e-3 nearest-neighbor distances. tanh scale compensates.
COORD_SCALE = np.float32(16.0)

F32 = mybir.dt.float32
F16 = mybir.dt.float16
MIN = mybir.AluOpType.min
MULT = mybir.AluOpType.mult
ADD = mybir.AluOpType.add
AX = mybir.AxisListType.X
AF = mybir.ActivationFunctionType

_nc_cache = []


def _build():
    nc = bacc.Bacc(
        "TRN2", target_bir_lowering=False, debug=False, num_devices=NCORES
    )
    lhsT_d = nc.declare_dram_parameter("lhsT", [BPC, T, 5, 128], F32, isOutput=False)
    rhs_d = nc.declare_dram_parameter("rhs", [BPC, 5, NH], F32, isOutput=False)
    mo_d = nc.declare_dram_parameter("mask_o", [128, BPC * T], F32, isOutput=False)
    me_d = nc.declare_dram_parameter("mask_ext", [128, BPC * HC], F32, isOutput=False)
    mi_d = nc.declare_dram_parameter("mask_int", [128, BPC * HC], F32, isOutput=False)
    out_d = nc.declare_dram_parameter("out", [128, 3], F32, isOutput=True)

    with ExitStack() as ctx:
        tc = ctx.enter_context(tile.TileContext(nc))
        singles = ctx.enter_context(tc.tile_pool(name="singles", bufs=1))
        augp = ctx.enter_context(tc.tile_pool(name="augp", bufs=2))
        rhp = ctx.enter_context(tc.tile_pool(name="rhp", bufs=2))
        slabp = ctx.enter_context(tc.tile_pool(name="slabp", bufs=2))
        s2p = ctx.enter_context(tc.tile_pool(name="s2p", bufs=2))
        scr1p = ctx.enter_context(tc.tile_pool(name="scr1p", bufs=2))
        scr2p = ctx.enter_context(tc.tile_pool(name="scr2p", bufs=2))
        maccp = ctx.enter_context(tc.tile_pool(name="maccp", bufs=2))
        psump = ctx.enter_context(tc.tile_pool(name="psump", bufs=3, space="PSUM"))
        tpp = ctx.enter_context(tc.tile_pool(name="tpp", bufs=2, space="PSUM"))

        ident = singles.tile([128, 128], F32)
        make_identity(nc, ident)

        MO = singles.tile([128, BPC * T], F16)  # per-obj-tile minima
        MH = singles.tile([128, BPC * HC], F32)  # assembled minho
        nc.vector.memset(MH, 0.0)

        for b in range(BPC):
            aug = augp.tile([5, T, 128], F32)
            nc.gpsimd.dma_start(out=aug, in_=lhsT_d[b].rearrange("t k m -> k t m"))
            rh = rhp.tile([5, NH], F32)
            nc.gpsimd.dma_start(out=rh, in_=rhs_d[b])
            s2 = s2p.tile([128, NG, NH], F16)
            for g in range(NG):
                slab = slabp.tile([128, G, NH], F16)
                for k in range(G):
                    t = g * G + k
                    ps = psump.tile([128, NH], F32)
                    nc.tensor.matmul(
                        ps[:, 0:512], aug[:, t, :], rh[:, 0:512],
                        start=True, stop=True,
                    )
                    nc.tensor.matmul(
                        ps[:, 512:NH], aug[:, t, :], rh[:, 512:NH],
                        start=True, stop=True,
                    )
                    nc.scalar.copy(slab[:, k, :], ps[:, :])  # drain f32->f16
                # minoh for these G obj tiles: [128, G]
                nc.vector.tensor_reduce(
                    MO[:, b * T + g * G : b * T + (g + 1) * G], slab[:, :, :],
                    axis=AX, op=MIN,
                )
                # fold tree: min over the G tiles -> s2[:, g, :]
                f1 = scr1p.tile([128, G // 2, NH], F16)
                nc.vector.tensor_tensor(f1, slab[:, 0 : G // 2, :], slab[:, G // 2 : G, :], MIN)
                f2 = scr2p.tile([128, G // 4, NH], F16)
                nc.vector.tensor_tensor(f2, f1[:, 0 : G // 4, :], f1[:, G // 4 : G // 2, :], MIN)
                nc.vector.tensor_tensor(s2[:, g, :], f2[:, 0, :], f2[:, 1, :], MIN)
            # fold across groups -> macc f32 [128, NH]
            g1 = scr1p.tile([128, NG // 2, NH], F16)
            nc.vector.tensor_tensor(g1, s2[:, 0 : NG // 2, :], s2[:, NG // 2 : NG, :], MIN)
            g2 = scr2p.tile([128, NG // 4, NH], F16)
            nc.vector.tensor_tensor(g2, g1[:, 0 : NG // 4, :], g1[:, NG // 4 : NG // 2, :], MIN)
            macc = maccp.tile([128, NH], F32)
            nc.vector.tensor_tensor(macc, g2[:, 0, :], g2[:, 1, :], MIN)
            # partition-axis min via PE transposes
            for c in range(HC):
                fc = min(128, NH - c * 128)
                tp = tpp.tile([128, 128], F32)
                nc.tensor.transpose(tp[0:fc, :], macc[:, c * 128 : c * 128 + fc], ident)
                nc.vector.tensor_reduce(
                    MH[0:fc, b * HC + c : b * HC + c + 1], tp[0:fc, :],
                    axis=AX, op=MIN,
                )

        # ---- end phase: pointwise + masked sums ----
        MOf = singles.tile([128, BPC * T], F32)
        nc.vector.tensor_copy(MOf, MO)
        nc.vector.tensor_scalar_max(MOf, MOf, 0.0)
        nc.vector.tensor_scalar_min(MOf, MOf, 1.0e4)
        nc.vector.tensor_scalar_max(MH, MH, 0.0)
        nc.vector.tensor_scalar_min(MH, MH, 1.0e4)
        nc.scalar.sqrt(MOf, MOf)
        nc.scalar.activation(MOf, MOf, AF.Tanh, scale=40.0 / float(COORD_SCALE))
        nc.scalar.sqrt(MH, MH)
        nc.scalar.activation(MH, MH, AF.Tanh, scale=40.0 / float(COORD_SCALE))

        mo_m = singles.tile([128, BPC * T], F32)
        nc.gpsimd.dma_start(out=mo_m, in_=mo_d[:, :])
        me_m = singles.tile([128, BPC * HC], F32)
        nc.gpsimd.dma_start(out=me_m, in_=me_d[:, :])
        mi_m = singles.tile([128, BPC * HC], F32)
        nc.gpsimd.dma_start(out=mi_m, in_=mi_d[:, :])

        outsb = singles.tile([128, 3], F32)
        junk_o = singles.tile([128, BPC * T], F32)
        junk_h = singles.tile([128, BPC * HC], F32)
        junk_h2 = singles.tile([128, BPC * HC], F32)
        nc.vector.tensor_tensor(junk_h, MH, me_m, MULT)
        nc.vector.tensor_reduce(outsb[:, 0:1], junk_h, axis=AX, op=ADD)
        nc.vector.tensor_tensor(junk_h2, MH, mi_m, MULT)
        nc.vector.tensor_reduce(outsb[:, 1:2], junk_h2, axis=AX, op=ADD)
        nc.vector.tensor_tensor(junk_o, MOf, mo_m, MULT)
        nc.vector.tensor_reduce(outsb[:, 2:3], junk_o, axis=AX, op=ADD)
        nc.sync.dma_start(out=out_d[:, :], in_=outsb)
    nc.compile()
    return nc


def _get_nc():
    if not _nc_cache:
        _nc_cache.append(_build())
    return _nc_cache[0]


def kernel(hand_verts, obj_verts, obj_split_sizes, exterior_hand, exterior_obj):
    hv = np.ascontiguousarray(hand_verts, dtype=np.float32) * COORD_SCALE  # [B, NH, 3]
    ov = np.ascontiguousarray(obj_verts, dtype=np.float32) * COORD_SCALE  # [B, NO, 3]
    splits = np.asarray(obj_split_sizes).astype(np.int64).reshape(B)
    eh = np.asarray(exterior_hand).astype(bool).reshape(B, NH)
    eo = np.asarray(exterior_obj).astype(bool).reshape(B, NO)

    xx = (hv * hv).sum(-1).astype(np.float32)  # [B, NH]
    yy = (ov * ov).sum(-1).astype(np.float32)  # [B, NO]
    valid = np.arange(NO)[None, :] < splits[:, None]
    yyM = (yy + BIG * (~valid)).astype(np.float32)

    # lhsT [B, T, 5, 128]: obj tile t holds obj verts j = t*128 + p
    o_t = ov.reshape(B, T, 128, 3).transpose(0, 1, 3, 2)
    lhsT = np.concatenate(
        [o_t, yyM.reshape(B, T, 1, 128), np.ones((B, T, 1, 128), np.float32)], axis=2
    ).astype(np.float32)
    # rhs [B, 5, NH]
    rhs = np.concatenate(
        [(-2.0 * hv).transpose(0, 2, 1), np.ones((B, 1, NH), np.float32), xx[:, None, :]],
        axis=1,
    ).astype(np.float32)
    # masks in device layouts
    mo = ((~eo) & valid).astype(np.float32).reshape(B, T, 128).transpose(0, 2, 1)
    ehp = np.zeros((B, HC * 128), np.float32)
    ehp[:, :NH] = eh
    ihp = np.zeros((B, HC * 128), np.float32)
    ihp[:, :NH] = ~eh
    me = ehp.reshape(B, HC, 128).transpose(0, 2, 1)  # [B, 128, HC]
    mi = ihp.reshape(B, HC, 128).transpose(0, 2, 1)

    in_maps = []
    for c in range(NCORES):
        bs = slice(c * BPC, (c + 1) * BPC)
        in_maps.append(
            {
                "lhsT": np.ascontiguousarray(lhsT[bs]),
                "rhs": np.ascontiguousarray(rhs[bs]),
                "mask_o": np.ascontiguousarray(
                    mo[bs].transpose(1, 0, 2).reshape(128, BPC * T)
                ),
                "mask_ext": np.ascontiguousarray(
                    me[bs].transpose(1, 0, 2).reshape(128, BPC * HC)
                ),
                "mask_int": np.ascontiguousarray(
                    mi[bs].transpose(1, 0, 2).reshape(128, BPC * HC)
                ),
            }
        )

    nc = _get_nc()
    res = run_bass_kernel_spmd(nc, in_maps, list(range(NCORES))).results

    nums = np.zeros(3, np.float64)
    for r in res:
        nums += r["out"].astype(np.float64).sum(axis=0)
    dens = np.array(
        [eh.sum(), (~eh).sum(), ((~eo) & valid).sum()], dtype=np.float64
    )
    out = np.where(dens > 0, 0.025 * nums / np.maximum(dens, 1.0), 0.0)
    return out.astype(np.float32)


# revision 9
# speedup vs baseline: 2.5826x; 2.5826x over previous
"""ContactLoss Trainium2 kernel (8 NeuronCores, batch data-parallel).

Math: all three losses only need per-hand-vertex and per-obj-vertex MIN
squared distances (the reference's argmin+gather+norm equals sqrt(min d2)),
followed by tanh/sqrt pointwise ops and masked means.

Per core (4 batches):
  d2 tile [128 obj (partitions), 778 hand (free)] computed on PE via K=5
  augmented matmul: rows(lhsT) = [o_x, o_y, o_z, yy+BIG*invalid, 1],
  rows(rhs) = [-2h_x, -2h_y, -2h_z, 1, xx]  ->  d2 = -2 o.h + yyM + xx.
  ACT drains PSUM -> SBUF f16 (enables DVE 2x modes), DVE does:
    - per-obj-tile row-min (minoh) via 3D tensor_reduce over tile groups
    - cross-tile elementwise-min tree (minho fold), then PE transposes +
      small reduces give the partition-axis min.
  tanh(sqrt(.)) on the tiny minima vectors, masked partial sums out.
Host: shard/augment inputs, sum partial numerators, divide by mask counts.
"""

import sys
from contextlib import ExitStack

import numpy as np

sys.path.insert(0, "/opt/trn_rl_repo")

import concourse.bass as bass  # noqa: E402
import concourse.mybir as mybir  # noqa: E402
import concourse.tile as tile  # noqa: E402
from concourse import bacc  # noqa: E402
from concourse.bass_utils import run_bass_kernel_spmd  # noqa: E402
from concourse.masks import make_identity  # noqa: E402

B, NH, NO = 32, 778, 3 * 8192 // 3  # 32, 778, 8192
NO = 8192
NCORES = 8
BPC = B // NCORES  # batches per core
T = NO // 128  # 64 obj tiles per batch
G = 8  # obj tiles per slab group
NG = T // G  # 8 groups per batch
HC = (NH + 127) // 128  # 7 hand column-chunks for the transpose stage
# Mask offset for invalid obj slots. Must dominate any real d2 (<= ~2 here)
# while staying finite in f16 (max 65504) so no inf/NaN enters the pipeline.
BIG = np.float32(49152.0)  # bf16-exact, >> max real d2 (~1100 after scaling)
PAD = np.float32(8192.0)  # hand-pad d2 offset; keeps all f16 sums finite
KD = 24  # split-K rows: 18 coord-product pairs + 3 yy + 3 xx
NHP = 784  # hand dim padded for 4B-aligned TT-tree halves
# Coordinate pre-scale: d2 values land in f16's normal range (>=6.1e-5)
# even for ~1e-3 nearest-neighbor distances. tanh scale compensates.
COORD_SCALE = np.float32(16.0)

F32 = mybir.dt.float32
F16 = mybir.dt.float16
BF16 = mybir.dt.bfloat16
MIN = mybir.AluOpType.min
MULT = mybir.AluOpType.mult
ADD = mybir.AluOpType.add
AX = mybir.AxisListType.X
AF = mybir.ActivationFunctionType

_nc_cache = []


def _build():
    nc = bacc.Bacc(
        "TRN2", target_bir_lowering=False, debug=False, num_devices=NCORES
    )
    lhsT_d = nc.declare_dram_parameter("lhsT", [BPC, T, KD, 128], BF16, isOutput=False)
    rhs_d = nc.declare_dram_parameter("rhs", [BPC, KD, NHP], BF16, isOutput=False)
    mo_d = nc.declare_dram_parameter("mask_o", [128, BPC * T], F32, isOutput=False)
    me_d = nc.declare_dram_parameter("mask_ext", [128, BPC * HC], F32, isOutput=False)
    mi_d = nc.declare_dram_parameter("mask_int", [128, BPC * HC], F32, isOutput=False)
    out_d = nc.declare_dram_parameter("out", [128, 3], F32, isOutput=True)

    with ExitStack() as ctx:
        tc = ctx.enter_context(tile.TileContext(nc))
        singles = ctx.enter_context(tc.tile_pool(name="singles", bufs=1))
        augp = ctx.enter_context(tc.tile_pool(name="augp", bufs=2))
        rhp = ctx.enter_context(tc.tile_pool(name="rhp", bufs=2))
        slabp = ctx.enter_context(tc.tile_pool(name="slabp", bufs=2))
        s2p = ctx.enter_context(tc.tile_pool(name="s2p", bufs=2))
        scr1p = ctx.enter_context(tc.tile_pool(name="scr1p", bufs=2))
        scr2p = ctx.enter_context(tc.tile_pool(name="scr2p", bufs=2))
        scr3p = ctx.enter_context(tc.tile_pool(name="scr3p", bufs=2))
        scr4p = ctx.enter_context(tc.tile_pool(name="scr4p", bufs=2))
        scr5p = ctx.enter_context(tc.tile_pool(name="scr5p", bufs=2))
        maccp = ctx.enter_context(tc.tile_pool(name="maccp", bufs=2))
        psump = ctx.enter_context(tc.tile_pool(name="psump", bufs=3, space="PSUM"))
        tpp = ctx.enter_context(tc.tile_pool(name="tpp", bufs=2, space="PSUM"))

        ident = singles.tile([128, 128], F32)
        make_identity(nc, ident)

        MO = singles.tile([128, BPC * T], F16)  # per-obj-tile minima
        MH = singles.tile([128, BPC * HC], F32)  # assembled minho
        nc.vector.memset(MH, 0.0)

        for b in range(BPC):
            aug = augp.tile([KD, T, 128], BF16)
            nc.gpsimd.dma_start(out=aug, in_=lhsT_d[b].rearrange("t k m -> k t m"))
            rh = rhp.tile([KD, NHP], BF16)
            nc.gpsimd.dma_start(out=rh, in_=rhs_d[b])
            s2 = s2p.tile([128, NG, NHP], F16)
            for g in range(NG):
                slab = slabp.tile([128, G, NHP], F16)
                for k in range(G):
                    t = g * G + k
                    ps = psump.tile([128, NHP], F32)
                    nc.tensor.matmul(
                        ps[:, 0:512], aug[:, t, :], rh[:, 0:512],
                        start=True, stop=True,
                    )
                    nc.tensor.matmul(
                        ps[:, 512:NHP], aug[:, t, :], rh[:, 512:NHP],
                        start=True, stop=True,
                    )
                    nc.scalar.copy(slab[:, k, :], ps[:, :])  # drain f32->f16
                # minoh for these G obj tiles: TT-tree (2x f16), then reduce
                s1t = scr3p.tile([128, G, 392], F16)
                nc.vector.tensor_tensor(s1t, slab[:, :, 0:392], slab[:, :, 392:NHP], MIN)
                s2t = scr4p.tile([128, G, 196], F16)
                nc.vector.tensor_tensor(s2t, s1t[:, :, 0:196], s1t[:, :, 196:392], MIN)
                s3t = scr5p.tile([128, G, 98], F16)
                nc.vector.tensor_tensor(s3t, s2t[:, :, 0:98], s2t[:, :, 98:196], MIN)
                nc.vector.tensor_reduce(
                    MO[:, b * T + g * G : b * T + (g + 1) * G], s3t[:, :, :],
                    axis=AX, op=MIN,
                )
                # fold tree: min over the G tiles -> s2[:, g, :]
                f1 = scr1p.tile([128, G // 2, NHP], F16)
                nc.vector.tensor_tensor(f1, slab[:, 0 : G // 2, :], slab[:, G // 2 : G, :], MIN)
                f2 = scr2p.tile([128, G // 4, NHP], F16)
                nc.vector.tensor_tensor(f2, f1[:, 0 : G // 4, :], f1[:, G // 4 : G // 2, :], MIN)
                nc.vector.tensor_tensor(s2[:, g, :], f2[:, 0, :], f2[:, 1, :], MIN)
            # fold across groups -> macc f32 [128, NH]
            g1 = scr1p.tile([128, NG // 2, NHP], F16)
            nc.vector.tensor_tensor(g1, s2[:, 0 : NG // 2, :], s2[:, NG // 2 : NG, :], MIN)
            g2 = scr2p.tile([128, NG // 4, NHP], F16)
            nc.vector.tensor_tensor(g2, g1[:, 0 : NG // 4, :], g1[:, NG // 4 : NG // 2, :], MIN)
            macc = maccp.tile([128, NHP], F32)
            nc.vector.tensor_tensor(macc, g2[:, 0, :], g2[:, 1, :], MIN)
            # partition-axis min via PE transposes
            for c in range(HC):
                fc = min(128, NHP - c * 128)
                tp = tpp.tile([128, 128], F32)
                nc.tensor.transpose(tp[0:fc, :], macc[:, c * 128 : c * 128 + fc], ident)
                nc.vector.tensor_reduce(
                    MH[0:fc, b * HC + c : b * HC + c + 1], tp[0:fc, :],
                    axis=AX, op=MIN,
                )

        # ---- end phase: pointwise + masked sums ----
        MOf = singles.tile([128, BPC * T], F32)
        nc.vector.tensor_copy(MOf, MO)
        nc.vector.tensor_scalar_max(MOf, MOf, 0.0)
        nc.vector.tensor_scalar_min(MOf, MOf, 1.0e4)
        nc.vector.tensor_scalar_max(MH, MH, 0.0)
        nc.vector.tensor_scalar_min(MH, MH, 1.0e4)
        nc.scalar.sqrt(MOf, MOf)
        nc.scalar.activation(MOf, MOf, AF.Tanh, scale=40.0 / float(COORD_SCALE))
        nc.scalar.sqrt(MH, MH)
        nc.scalar.activation(MH, MH, AF.Tanh, scale=40.0 / float(COORD_SCALE))

        mo_m = singles.tile([128, BPC * T], F32)
        nc.gpsimd.dma_start(out=mo_m, in_=mo_d[:, :])
        me_m = singles.tile([128, BPC * HC], F32)
        nc.gpsimd.dma_start(out=me_m, in_=me_d[:, :])
        mi_m = singles.tile([128, BPC * HC], F32)
        nc.gpsimd.dma_start(out=mi_m, in_=mi_d[:, :])

        outsb = singles.tile([128, 3], F32)
        junk_o = singles.tile([128, BPC * T], F32)
        junk_h = singles.tile([128, BPC * HC], F32)
        junk_h2 = singles.tile([128, BPC * HC], F32)
        nc.vector.tensor_tensor(junk_h, MH, me_m, MULT)
        nc.vector.tensor_reduce(outsb[:, 0:1], junk_h, axis=AX, op=ADD)
        nc.vector.tensor_tensor(junk_h2, MH, mi_m, MULT)
        nc.vector.tensor_reduce(outsb[:, 1:2], junk_h2, axis=AX, op=ADD)
        nc.vector.tensor_tensor(junk_o, MOf, mo_m, MULT)
        nc.vector.tensor_reduce(outsb[:, 2:3], junk_o, axis=AX, op=ADD)
        nc.sync.dma_start(out=out_d[:, :], in_=outsb)
    nc.compile()
    return nc


def _get_nc():
    if not _nc_cache:
        _nc_cache.append(_build())
    return _nc_cache[0]


def kernel(hand_verts, obj_verts, obj_split_sizes, exterior_hand, exterior_obj):
    hv = np.ascontiguousarray(hand_verts, dtype=np.float32) * COORD_SCALE  # [B, NH, 3]
    ov = np.ascontiguousarray(obj_verts, dtype=np.float32) * COORD_SCALE  # [B, NO, 3]
    splits = np.asarray(obj_split_sizes).astype(np.int64).reshape(B)
    eh = np.asarray(exterior_hand).astype(bool).reshape(B, NH)
    eo = np.asarray(exterior_obj).astype(bool).reshape(B, NO)

    xx = (hv * hv).sum(-1).astype(np.float32)  # [B, NH]
    yy = (ov * ov).sum(-1).astype(np.float32)  # [B, NO]
    valid = np.arange(NO)[None, :] < splits[:, None]
    yyM = (yy + BIG * (~valid)).astype(np.float32)

    import ml_dtypes

    def split3(x):
        x0 = x.astype(ml_dtypes.bfloat16).astype(np.float32)
        r = x - x0
        x1 = r.astype(ml_dtypes.bfloat16).astype(np.float32)
        x2 = r - x1
        return x0, x1, x2

    o0, o1, o2 = split3(ov)  # each [B, NO, 3], bf16-exact values
    h0, h1, h2 = split3(hv)
    y0, y1, y2 = split3(yyM)
    x0, x1, x2 = split3(xx)
    # product pairs (obj_part, hand_part): exact o.h to ~2^-26
    A_SEQ = [o0, o0, o1, o1, o0, o2]
    B_SEQ = [h0, h1, h0, h1, h2, h0]
    # lhsT [B, T, KD, 128]
    obj_rows = np.stack(A_SEQ, axis=2).reshape(B, NO, 18)  # [B,NO,6,3]->[B,NO,18]
    y_rows = np.stack([y0, y1, y2], axis=2)  # [B, NO, 3]
    ones_o = np.ones((B, NO, 3), np.float32)
    lhsT = (
        np.concatenate([obj_rows, y_rows, ones_o], axis=2)
        .reshape(B, T, 128, KD)
        .transpose(0, 1, 3, 2)
        .astype(ml_dtypes.bfloat16)
    )
    # rhs [B, KD, NHP]: hand pads get xx=PAD so padded d2 is large but finite
    h_rows = np.stack([-2.0 * h for h in B_SEQ], axis=2).reshape(B, NH, 18)
    ones_h = np.ones((B, NH, 3), np.float32)
    x_rows = np.stack([x0, x1, x2], axis=2)  # [B, NH, 3]
    rhs_core = np.concatenate([h_rows, ones_h, x_rows], axis=2).transpose(0, 2, 1)
    rhs = np.zeros((B, KD, NHP), np.float32)
    rhs[:, :, :NH] = rhs_core
    rhs[:, 21, NH:] = PAD  # x0 row at pad columns
    rhs = rhs.astype(ml_dtypes.bfloat16)
    # masks in device layouts
    mo = ((~eo) & valid).astype(np.float32).reshape(B, T, 128).transpose(0, 2, 1)
    ehp = np.zeros((B, HC * 128), np.float32)
    ehp[:, :NH] = eh
    ihp = np.zeros((B, HC * 128), np.float32)
    ihp[:, :NH] = ~eh
    me = ehp.reshape(B, HC, 128).transpose(0, 2, 1)  # [B, 128, HC]
    mi = ihp.reshape(B, HC, 128).transpose(0, 2, 1)

    in_maps = []
    for c in range(NCORES):
        bs = slice(c * BPC, (c + 1) * BPC)
        in_maps.append(
            {
                "lhsT": np.ascontiguousarray(lhsT[bs]),
                "rhs": np.ascontiguousarray(rhs[bs]),
                "mask_o": np.ascontiguousarray(
                    mo[bs].transpose(1, 0, 2).reshape(128, BPC * T)
                ),
                "mask_ext": np.ascontiguousarray(
                    me[bs].transpose(1, 0, 2).reshape(128, BPC * HC)
                ),
                "mask_int": np.ascontiguousarray(
                    mi[bs].transpose(1, 0, 2).reshape(128, BPC * HC)
                ),
            }
        )

    nc = _get_nc()
    res = run_bass_kernel_spmd(nc, in_maps, list(range(NCORES))).results

    nums = np.zeros(3, np.float64)
    for r in res:
        nums += r["out"].astype(np.float64).sum(axis=0)
    dens = np.array(
        [eh.sum(), (~eh).sum(), ((~eo) & valid).sum()], dtype=np.float64
    )
    out = np.where(dens > 0, 0.025 * nums / np.maximum(dens, 1.0), 0.0)
    return out.astype(np.float32)
